# revision 1
# baseline (speedup 1.0000x reference)
"""CBOW negative-sampling loss kernel for 8 TRN2 NeuronCores.

Strategy (data-parallel, per sharding hint):
  - Shard the batch (B=16384) across 8 cores -> 2048 rows/core.
  - Per core the embedding tables are COMPACTED on host: only the
    distinct rows this core's lookups touch (<=22528, worst case) are
    uploaded, relabelled 0..n-1, padded to 32768 rows, bf16.  This
    more than halves HBM footprint vs replicating the full f32 tables
    and lets indices fit int16.
  - The 43008 row-gathers per core are split between the two SWDGE
    descriptor-generation paths, which run CONCURRENTLY on different
    queues:
      * qPoolDynamic(0):  classic indirect_dma_start, one index per
        partition per instruction (~1.5us / 128 rows measured).
      * qPoolDynamic1..3: batched dma_gather ucode (int16 index list,
        ~25ns/row single-queue, ~2.3x faster across 3 queues).
    Both are slot-exact (dma_gather places list element j at partition
    j%128, block j//128), so the compute is identical for all tiles
    and the target always lands at candidate 0.
  - DVE computes per-tile context sums (add tree) and the 11 dot
    products per row (mult + segmented reduce) in bf16; ACT applies
    sigmoid to ALL scores with scale -0.1 and a single ln(+eps) with
    free-dim accumulation.  The target's positive term is recovered on
    host via log sig(x) - log sig(-x) = x, i.e. loss row-sum =
    sum_c log(sig(-s_c/10)+eps) + s_pos/10.
"""

import os

import numpy as np

import concourse.bacc as bacc
import concourse.bass as bass
import concourse.mybir as mybir
import concourse.tile as tile
from concourse.bass_utils import run_bass_kernel_spmd

VOCAB = 100000
DIM = 128
B = 16384
CWIN = 10
K = 10
EPS = 1e-9
NCORES = 8
P = 128
BPC = B // NCORES            # 2048 batch rows per core
NTILES = BPC // P            # 16 tiles of 128 rows
NIDX = CWIN + 1 + K          # 21 lookups per batch row
CTAB = 32768                 # compacted table rows (per core, per table)

F32 = mybir.dt.float32
BF16 = mybir.dt.bfloat16
I16 = mybir.dt.int16
I32 = mybir.dt.int32
MULT = mybir.AluOpType.mult
ADD = mybir.AluOpType.add
AX_X = mybir.AxisListType.X
SIGMOID = mybir.ActivationFunctionType.Sigmoid
LN = mybir.ActivationFunctionType.Ln

# ---- tunables -----------------------------------------------------------
# Measured on HW: the qPoolDynamic descriptor-generation ucode is the
# bottleneck (~1.3us serial per 128-row indirect instruction; the
# batched dma_gather ucode is no faster per row in-kernel and mixing
# the two paths interferes).  All-indirect measured best.
NG_TILES = int(os.environ.get("KCFG_NG", "0"))   # tiles via dma_gather
GCHUNK = int(os.environ.get("KCFG_GCHUNK", "2")) # tiles per dma_gather inst
NQUEUES = int(os.environ.get("KCFG_NQUEUES", "4"))
GATHER_BUFS = int(os.environ.get("KCFG_GBUFS", "4"))
IND_BUFS = int(os.environ.get("KCFG_IBUFS", "3"))
USE_TTR = os.environ.get("KCFG_TTR", "0") == "1"

NI_TILES = NTILES - NG_TILES                     # tiles via indirect path
assert NG_TILES % GCHUNK == 0
NGC = NG_TILES // GCHUNK                         # dma_gather chunk count
CTX_NI = GCHUNK * CWIN * P                       # idx per ctx gather (2560)
TN_NI = GCHUNK * (K + 1) * P                     # idx per tn gather (2816)
CW16 = CTX_NI // 16
TW16 = TN_NI // 16
CHUNK16 = CW16 + TW16


def build_kernel_body(tc, idx32, idx16, ctab_in, ctab_out, usum):
    """Emit the per-core program.

    idx32: [P, max(NI_TILES,1)*NIDX] int32. For indirect tile u (global
           tile t = NG_TILES+u), cols u*21+j: j<10 ctx lookups
           (compacted in_emb ids), j>=10 target+negatives (compacted
           out_emb ids; j==10 is the target).
    idx16: [P, max(NGC,1)*CHUNK16] int16 dma_gather wrapped lists per
           chunk: ctx list (CTX_NI) then tn list (TN_NI); list elem j
           -> partition j%128, block j//128; blocks ordered
           (tile-in-chunk, slot), target at slot 0 of each tn group.
    usum:  [P, 2] f32; col 0 = sum over all tiles/candidates of
           log(sigmoid(-s/10)+eps); col 1 = sum over tiles of raw
           target score s_pos.
    """
    nc = tc.nc
    with (
        tc.tile_pool(name="io", bufs=1) as io_pool,
        tc.tile_pool(name="g", bufs=GATHER_BUFS) as gpool,
        tc.tile_pool(name="ind", bufs=IND_BUFS) as ipool,
        tc.tile_pool(name="work", bufs=2) as wpool,
    ):
        idx32_t = io_pool.tile([P, max(NI_TILES, 1) * NIDX], I32)
        if NI_TILES:
            nc.sync.dma_start(out=idx32_t[:], in_=idx32[:, :])
        idx16_t = io_pool.tile([P, max(NGC, 1) * CHUNK16], I16)
        if NGC:
            nc.sync.dma_start(out=idx16_t[:], in_=idx16[:, :])

        eps_t = io_pool.tile([P, 1], F32)
        nc.vector.memset(eps_t[:], EPS)

        # staging for all scores; col t*11+c = candidate c of tile t
        s_all = io_pool.tile([P, NTILES * (K + 1)], F32)
        us = io_pool.tile([P, 2], F32)

        def compute_tile(t_idx, ctx_ap, tn_ap):
            a1 = wpool.tile([P, 5 * DIM], BF16, tag="a1")
            nc.vector.tensor_add(
                a1[:], ctx_ap[:, 0 : 5 * DIM], ctx_ap[:, 5 * DIM : 10 * DIM]
            )
            b1 = wpool.tile([P, 2 * DIM], BF16, tag="b1")
            nc.vector.tensor_add(
                b1[:], a1[:, 0 : 2 * DIM], a1[:, 2 * DIM : 4 * DIM]
            )
            cs = wpool.tile([P, DIM], BF16, tag="cs")
            nc.vector.tensor_add(cs[:], b1[:, 0:DIM], b1[:, DIM : 2 * DIM])
            nc.vector.tensor_add(cs[:], cs[:], a1[:, 4 * DIM : 5 * DIM])

            prod = wpool.tile([P, (K + 1) * DIM], BF16, tag="prod")
            if USE_TTR:
                # fused (tn*cs) multiply + full reduce per candidate:
                # two-source op, avoids the single-src 2-port DVE mode
                # that locks GpSimd out of the SWDGE descriptor rings.
                for k in range(K + 1):
                    nc.vector.tensor_tensor_reduce(
                        out=prod[:, k * DIM : (k + 1) * DIM],
                        in0=tn_ap[:, k * DIM : (k + 1) * DIM],
                        in1=cs[:],
                        scale=1.0,
                        scalar=0.0,
                        op0=MULT,
                        op1=ADD,
                        accum_out=s_all[
                            :, t_idx * (K + 1) + k : t_idx * (K + 1) + k + 1
                        ],
                    )
            else:
                prod3 = prod[:].rearrange("p (k d) -> p k d", d=DIM)
                tn3 = tn_ap.rearrange("p (k d) -> p k d", d=DIM)
                cs_b = cs[:][:, None, :].to_broadcast([P, K + 1, DIM])
                nc.vector.tensor_tensor(prod3, tn3, cs_b, MULT)
                nc.vector.tensor_reduce(
                    out=s_all[:, t_idx * (K + 1) : (t_idx + 1) * (K + 1)],
                    in_=prod3, axis=AX_X, op=ADD,
                )

        # ---- dma_gather tiles: phase-grouped (all ctx gathers, then all
        # tn gathers -> one in_ap table switch), single queue, one pool
        # tile per instruction with bufs=NGC so nothing is reused
        # (zero WAR edges); compute trails after the tn phase.
        ctx_gs, tn_gs = [], []
        for c in range(NGC):
            base16 = c * CHUNK16
            ctx_g = gpool.tile([P, GCHUNK * CWIN * DIM], BF16, tag="gctx")
            nc.gpsimd.dma_gather(
                out_ap=ctx_g[:].rearrange("p (q d) -> p q d", d=DIM),
                in_ap=ctab_in[:, :],
                idxs_ap=idx16_t[:, base16 : base16 + CW16],
                num_idxs=CTX_NI,
                num_idxs_reg=CTX_NI,
                elem_size=DIM,
                single_packet=False,
                queue_num=0,
            )
            ctx_gs.append(ctx_g)
        for c in range(NGC):
            base16 = c * CHUNK16
            tn_g = gpool.tile([P, GCHUNK * (K + 1) * DIM], BF16, tag="gtn")
            nc.gpsimd.dma_gather(
                out_ap=tn_g[:].rearrange("p (q d) -> p q d", d=DIM),
                in_ap=ctab_out[:, :],
                idxs_ap=idx16_t[:, base16 + CW16 : base16 + CHUNK16],
                num_idxs=TN_NI,
                num_idxs_reg=TN_NI,
                elem_size=DIM,
                single_packet=False,
                queue_num=0,
            )
            tn_gs.append(tn_g)
        for c in range(NGC):
            for b in range(GCHUNK):
                compute_tile(
                    c * GCHUNK + b,
                    ctx_gs[c][:, b * CWIN * DIM : (b + 1) * CWIN * DIM],
                    tn_gs[c][:, b * (K + 1) * DIM : (b + 1) * (K + 1) * DIM],
                )

        # ---- indirect tiles, queue 0.  Every slot-gather writes its OWN
        # pool tile so no two DMA instructions share a destination tile
        # (same-tile writers get semaphore-serialized by Tile); the Pool
        # instruction stream is then dependency-free apart from pool
        # rotation, which DVE consumption keeps ahead of.
        for u in range(NI_TILES):
            t_idx = NG_TILES + u
            c0 = u * NIDX
            ctx_t = []
            for j in range(CWIN):
                g = ipool.tile([P, DIM], BF16, tag=f"ic{j}")
                nc.gpsimd.indirect_dma_start(
                    out=g[:],
                    out_offset=None,
                    in_=ctab_in[:, :],
                    in_offset=bass.IndirectOffsetOnAxis(
                        ap=idx32_t[:, c0 + j : c0 + j + 1], axis=0
                    ),
                )
                ctx_t.append(g)
            tn_t = []
            for j in range(K + 1):
                g = ipool.tile([P, DIM], BF16, tag=f"it{j}")
                nc.gpsimd.indirect_dma_start(
                    out=g[:],
                    out_offset=None,
                    in_=ctab_out[:, :],
                    in_offset=bass.IndirectOffsetOnAxis(
                        ap=idx32_t[:, c0 + CWIN + j : c0 + CWIN + j + 1],
                        axis=0,
                    ),
                )
                tn_t.append(g)

            # context sum: 5 pairwise adds into a1 (DVE-only writers,
            # program-ordered, no semaphores), then the tree.
            a1 = wpool.tile([P, 5 * DIM], BF16, tag="a1")
            for j in range(5):
                nc.vector.tensor_add(
                    a1[:, j * DIM : (j + 1) * DIM],
                    ctx_t[j][:], ctx_t[j + 5][:],
                )
            b1 = wpool.tile([P, 2 * DIM], BF16, tag="b1")
            nc.vector.tensor_add(
                b1[:], a1[:, 0 : 2 * DIM], a1[:, 2 * DIM : 4 * DIM]
            )
            cs = wpool.tile([P, DIM], BF16, tag="cs")
            nc.vector.tensor_add(cs[:], b1[:, 0:DIM], b1[:, DIM : 2 * DIM])
            nc.vector.tensor_add(cs[:], cs[:], a1[:, 4 * DIM : 5 * DIM])

            # scores: per-candidate unit-stride multiplies (no broadcast
            # AP -> eligible for packed bf16 DVE modes), one segmented
            # reduce.
            prod = wpool.tile([P, (K + 1) * DIM], BF16, tag="prod")
            if USE_TTR:
                for k in range(K + 1):
                    nc.vector.tensor_tensor_reduce(
                        out=prod[:, k * DIM : (k + 1) * DIM],
                        in0=tn_t[k][:],
                        in1=cs[:],
                        scale=1.0,
                        scalar=0.0,
                        op0=MULT,
                        op1=ADD,
                        accum_out=s_all[
                            :, t_idx * (K + 1) + k : t_idx * (K + 1) + k + 1
                        ],
                    )
            else:
                for k in range(K + 1):
                    nc.vector.tensor_tensor(
                        prod[:, k * DIM : (k + 1) * DIM], tn_t[k][:], cs[:],
                        MULT,
                    )
                nc.vector.tensor_reduce(
                    out=s_all[:, t_idx * (K + 1) : (t_idx + 1) * (K + 1)],
                    in_=prod[:].rearrange("p (k d) -> p k d", d=DIM),
                    axis=AX_X, op=ADD,
                )

        # ---- batched activation phases (one table load each) ---------
        sig = io_pool.tile([P, NTILES * (K + 1)], F32)
        nc.scalar.activation(sig[:], s_all[:], SIGMOID, scale=-0.1)
        lnv = io_pool.tile([P, NTILES * (K + 1)], F32)
        nc.scalar.activation(
            lnv[:], sig[:], LN, bias=eps_t[:], accum_out=us[:, 0:1]
        )
        # sum of raw target scores (candidate 0 of each tile)
        spos = (
            s_all[:]
            .rearrange("p (t c) -> p t c", c=K + 1)[:, :, 0:1]
            .rearrange("p t c -> p (t c)")
        )
        nc.vector.tensor_reduce(out=us[:, 1:2], in_=spos, axis=AX_X, op=ADD)

        nc.sync.dma_start(out=usum[:, :], in_=us[:])


def build_nc():
    nc = bacc.Bacc(
        "TRN2",
        target_bir_lowering=False,
        debug=False,
        enable_asserts=False,
        num_devices=NCORES,
        num_swdge_queues=NQUEUES,
    )
    idx32 = nc.dram_tensor(
        "idx32", [P, max(NI_TILES, 1) * NIDX], I32, kind="ExternalInput"
    )
    idx16 = nc.dram_tensor(
        "idx16", [P, max(NGC, 1) * CHUNK16], I16, kind="ExternalInput"
    )
    ctab_in = nc.dram_tensor("ctab_in", [CTAB, DIM], BF16,
                             kind="ExternalInput")
    ctab_out = nc.dram_tensor("ctab_out", [CTAB, DIM], BF16,
                              kind="ExternalInput")
    usum = nc.dram_tensor("usum", [P, 2], F32, kind="ExternalOutput")
    with tile.TileContext(nc) as tc:
        build_kernel_body(tc, idx32.ap(), idx16.ap(), ctab_in.ap(),
                          ctab_out.ap(), usum.ap())
    nc.compile()
    return nc


def _wrap16(arr):
    """flat index list -> [128, n/16] int16 dma_gather layout."""
    w = arr.reshape(-1, 16).T
    return np.tile(w, (8, 1)).astype(np.int16)


def make_in_maps(context, target, negatives, in_emb, out_emb):
    context = np.asarray(context).astype(np.int64)
    target = np.asarray(target).astype(np.int64)
    negatives = np.asarray(negatives).astype(np.int64)
    in_emb = np.asarray(in_emb, dtype=np.float32)
    out_emb = np.asarray(out_emb, dtype=np.float32)
    bf16 = mybir.dt.np(BF16)
    tn_full = np.concatenate([target[:, None], negatives], axis=1)  # [B, 11]
    in_maps = []
    for c in range(NCORES):
        sl = slice(c * BPC, (c + 1) * BPC)
        # [P, NTILES, slots] index cubes (partition = batch row % 128)
        ctx_t = (
            context[sl].reshape(NTILES, P, CWIN).transpose(1, 0, 2)
        )  # [P, T, 10]
        tn_t = (
            tn_full[sl].reshape(NTILES, P, K + 1).transpose(1, 0, 2)
        )  # [P, T, 11], slot 0 = target

        # per-(core, table) compaction; <=20480/22528 distinct always
        uin, cin = np.unique(ctx_t, return_inverse=True)
        cin = cin.reshape(ctx_t.shape).astype(np.int32)
        uout, ctn = np.unique(tn_t, return_inverse=True)
        ctn = ctn.reshape(tn_t.shape).astype(np.int32)
        assert len(uin) <= CTAB and len(uout) <= CTAB
        ctab_in = np.zeros((CTAB, DIM), dtype=bf16)
        ctab_in[: len(uin)] = in_emb[uin].astype(bf16)
        ctab_out = np.zeros((CTAB, DIM), dtype=bf16)
        ctab_out[: len(uout)] = out_emb[uout].astype(bf16)

        # int16 dma_gather lists: chunk c covers tiles [2c, 2c+1];
        # list elem j -> (partition j%128, block j//128), block =
        # (tile_in_chunk, slot) -> order (b, w, p) when flattening
        parts16 = []
        for ch in range(NGC):
            t0 = ch * GCHUNK
            ctx_list = (
                cin[:, t0 : t0 + GCHUNK, :].transpose(1, 2, 0).reshape(-1)
            )
            tn_list = (
                ctn[:, t0 : t0 + GCHUNK, :].transpose(1, 2, 0).reshape(-1)
            )
            parts16.append(_wrap16(ctx_list))
            parts16.append(_wrap16(tn_list))
        if parts16:
            idx16 = np.ascontiguousarray(np.concatenate(parts16, axis=1))
        else:
            idx16 = np.zeros((P, CHUNK16), np.int16)

        # int32 indirect columns for tiles NG_TILES..15
        if NI_TILES:
            cols = np.concatenate(
                [cin[:, NG_TILES:, :], ctn[:, NG_TILES:, :]], axis=2
            )  # [P, NI_TILES, 21]
            idx32 = np.ascontiguousarray(
                cols.reshape(P, NI_TILES * NIDX).astype(np.int32)
            )
        else:
            idx32 = np.zeros((P, NIDX), np.int32)

        in_maps.append(
            {
                "idx32": idx32,
                "idx16": idx16,
                "ctab_in": ctab_in,
                "ctab_out": ctab_out,
            }
        )
    return in_maps


_NC_CACHE = []
LAST_RESULT = None  # BassKernelResults of the most recent run (for profiling)


def kernel(**inputs) -> np.ndarray:
    global LAST_RESULT
    in_maps = make_in_maps(
        inputs["context"],
        inputs["target"],
        inputs["negatives"],
        inputs["in_emb"],
        inputs["out_emb"],
    )
    if not _NC_CACHE:
        _NC_CACHE.append(build_nc())
    nc = _NC_CACHE[0]
    res = run_bass_kernel_spmd(nc, in_maps, core_ids=list(range(NCORES)))
    LAST_RESULT = res
    total = 0.0
    for r in res.results:
        u = r["usum"].astype(np.float64)
        total += u[:, 0].sum() + 0.1 * u[:, 1].sum()
    return np.array(-total / B, dtype=np.float32)



# revision 3
# speedup vs baseline: 1.7493x; 1.7493x over previous
"""CBOW negative-sampling loss kernel for 8 TRN2 NeuronCores.

Strategy (data-parallel, per sharding hint):
  - Shard the batch (B=16384) across 8 cores -> 2048 rows/core.
  - Per core the embedding tables are COMPACTED on host: only the
    distinct rows this core's lookups touch (<=20480 for in_emb,
    <=22528 for out_emb) are uploaded, relabelled 0..n-1, bf16.
    Indices fit int16.
  - ALL 43008 row-gathers per core go through batched dma_gather
    ucode in 11 large chunks (4096/2048 indices each) spread
    round-robin over 4 SWDGE queues.  Measured on HW: 4-queue
    dma_gather sustains the full 43008-row / 11 MB gather in ~30 us
    (HBM roofline), vs ~285 us single-queue (descriptor-gen bound)
    and ~440 us for the per-128-row indirect_dma_start path the
    previous version used.
  - Gather lists are SLOT-MAJOR: ctx chunk c carries context slots
    {2c, 2c+1} for all 16 tiles; tn chunk c carries candidates
    {2c, 2c+1} (candidate 0 = target).  Block layout per chunk is
    [local_slot(2) x tile(16) x dim(128)], so the context-sum tree
    and the per-candidate score reductions each run as a handful of
    2048..4096-column DVE instructions (no per-tile small ops).
  - DVE computes the 10-way context sum with a 9-add binary tree in
    bf16, then per candidate k one 2048-col multiply + one segmented
    f32 reduce -> s_all[P, (k,t)].
  - ACT applies sigmoid to ALL scores with scale -0.1 (folds the
    1/10 context-mean normalisation and the negation) and a single
    ln(+eps) with free-dim accumulation.  The target's positive term
    is recovered on host via log sig(x) - log sig(-x) = x, i.e.
    loss row-sum = sum_c log(sig(-s_c/10)+eps) + s_pos/10.
"""

import numpy as np

import concourse.bacc as bacc
import concourse.bass as bass
import concourse.mybir as mybir
import concourse.tile as tile
from concourse.bass_utils import run_bass_kernel_spmd

VOCAB = 100000
DIM = 128
B = 16384
CWIN = 10
K = 10
EPS = 1e-9
NCORES = 8
P = 128
BPC = B // NCORES            # 2048 batch rows per core
NTILES = BPC // P            # 16 tiles of 128 rows
CT_IN = 20480                # compacted in_emb rows (= 2048*10 worst case)
CT_OUT = 22528               # compacted out_emb rows (= 2048*11 worst case)
NQUEUES = 4

CTX_N = BPC * CWIN           # 20480 ctx lookups per core
TN_N = BPC * (K + 1)         # 22528 target+negative lookups per core
CHUNK = 4096                 # gather-list chunk (= 2 slots x 2048 rows)
NCH_CTX = CTX_N // CHUNK     # 5
NCH_TN = (TN_N + CHUNK - 1) // CHUNK   # 6 (last chunk is 2048)

F32 = mybir.dt.float32
BF16 = mybir.dt.bfloat16
I16 = mybir.dt.int16
MULT = mybir.AluOpType.mult
ADD = mybir.AluOpType.add
AX_X = mybir.AxisListType.X
SIGMOID = mybir.ActivationFunctionType.Sigmoid
LN = mybir.ActivationFunctionType.Ln


def build_kernel_body(tc, ctxidx, tnidx, ctab_in, ctab_out, usum, R=1):
    """Emit the per-core program.

    ctxidx: [P, NCH_CTX*256] int16 wrapped dma_gather lists; chunk c
            (columns 256c..256c+255) gathers context slots {2c, 2c+1}
            of all 16 tiles from ctab_in: list position
            j'*2048 + t*128 + p  ->  cin[p, t, 2c+j'].
    tnidx:  [P, 1408] int16; chunk c gathers candidates {2c, 2c+1}
            (c=5: candidate 10 only) from ctab_out; candidate 0 is
            the target.
    usum:   [P, 2] f32; col 0 = sum over candidates/tiles of
            log(sig(-s/10)+eps); col 1 = sum over tiles of raw
            target score (context-SUM dot target, no 1/10).
    """
    nc = tc.nc
    with (
        tc.tile_pool(name="io", bufs=1) as io_pool,
        tc.tile_pool(name="g", bufs=1) as gpool,
        tc.tile_pool(name="w", bufs=1) as wpool,
    ):
        ctxidx_t = io_pool.tile([P, NCH_CTX * (CHUNK // 16)], I16)
        nc.sync.dma_start(out=ctxidx_t[:], in_=ctxidx[:, :])
        tnidx_t = io_pool.tile([P, (TN_N // 16)], I16)
        nc.sync.dma_start(out=tnidx_t[:], in_=tnidx[:, :])

        eps_t = io_pool.tile([P, 1], F32)
        nc.vector.memset(eps_t[:], EPS)

        s_all = io_pool.tile([P, NTILES * (K + 1)], F32)
        us = io_pool.tile([P, 2], F32)

        for r in range(R):
            # ---- gathers: 11 chunks, round-robin over 4 queues ----
            q = 0
            ctx_g = []
            for c in range(NCH_CTX):
                t = gpool.tile([P, (CHUNK // P) * DIM], BF16, tag=f"gc{c}")
                nc.gpsimd.dma_gather(
                    out_ap=t[:].rearrange("p (q d) -> p q d", d=DIM),
                    in_ap=ctab_in[:, :],
                    idxs_ap=ctxidx_t[
                        :, c * (CHUNK // 16):(c + 1) * (CHUNK // 16)
                    ],
                    num_idxs=CHUNK,
                    num_idxs_reg=CHUNK,
                    elem_size=DIM,
                    single_packet=False,
                    queue_num=q % NQUEUES,
                )
                q += 1
                ctx_g.append(t)
            tn_g = []
            for c in range(NCH_TN):
                n = min(CHUNK, TN_N - c * CHUNK)
                t = gpool.tile([P, (n // P) * DIM], BF16, tag=f"gt{c}")
                nc.gpsimd.dma_gather(
                    out_ap=t[:].rearrange("p (q d) -> p q d", d=DIM),
                    in_ap=ctab_out[:, :],
                    idxs_ap=tnidx_t[:, c * (CHUNK // 16):
                                    c * (CHUNK // 16) + n // 16],
                    num_idxs=n,
                    num_idxs_reg=n,
                    elem_size=DIM,
                    single_packet=False,
                    queue_num=q % NQUEUES,
                )
                q += 1
                tn_g.append(t)

            # ---- context-sum tree (9 adds, bf16, big spans) ----
            W = NTILES * DIM  # 2048 cols per slot
            t1 = []
            for c in range(NCH_CTX):
                t = wpool.tile([P, W], BF16, tag=f"t1{c}")
                nc.vector.tensor_add(t[:], ctx_g[c][:, 0:W], ctx_g[c][:, W:2 * W])
                t1.append(t)
            t2a = wpool.tile([P, W], BF16, tag="t2a")
            nc.vector.tensor_add(t2a[:], t1[0][:], t1[1][:])
            t2b = wpool.tile([P, W], BF16, tag="t2b")
            nc.vector.tensor_add(t2b[:], t1[2][:], t1[3][:])
            cs = wpool.tile([P, W], BF16, tag="cs")
            nc.vector.tensor_add(cs[:], t2a[:], t2b[:])
            nc.vector.tensor_add(cs[:], cs[:], t1[4][:])
            cs3 = cs[:].rearrange("p (t d) -> p t d", d=DIM)

            # ---- scores: per candidate one mult + one segmented reduce
            prod = wpool.tile([P, W], BF16, tag="prod")
            prod3 = prod[:].rearrange("p (t d) -> p t d", d=DIM)
            for k in range(K + 1):
                src = tn_g[k // 2][:, (k % 2) * W:(k % 2) * W + W]
                nc.vector.tensor_tensor(prod3, src.rearrange(
                    "p (t d) -> p t d", d=DIM), cs3, MULT)
                nc.vector.tensor_reduce(
                    out=s_all[:, k * NTILES:(k + 1) * NTILES],
                    in_=prod3, axis=AX_X, op=ADD,
                )

            # ---- target raw-score row sum ------------------------
            nc.vector.tensor_reduce(
                out=us[:, 1:2], in_=s_all[:, 0:NTILES], axis=AX_X, op=ADD,
            )
            # ---- batched activation (one table load each) --------
            sig = io_pool.tile([P, NTILES * (K + 1)], F32)
            nc.scalar.activation(sig[:], s_all[:], SIGMOID, scale=-0.1)
            lnv = io_pool.tile([P, NTILES * (K + 1)], F32)
            nc.scalar.activation(
                lnv[:], sig[:], LN, bias=eps_t[:], accum_out=us[:, 0:1]
            )

        nc.sync.dma_start(out=usum[:, :], in_=us[:])


def build_nc(R=1):
    nc = bacc.Bacc(
        "TRN2",
        target_bir_lowering=False,
        debug=False,
        enable_asserts=False,
        num_devices=NCORES,
        num_swdge_queues=NQUEUES,
    )
    ctxidx = nc.dram_tensor(
        "ctxidx", [P, CTX_N // 16], I16, kind="ExternalInput"
    )
    tnidx = nc.dram_tensor(
        "tnidx", [P, TN_N // 16], I16, kind="ExternalInput"
    )
    ctab_in = nc.dram_tensor("ctab_in", [CT_IN, DIM], BF16,
                             kind="ExternalInput")
    ctab_out = nc.dram_tensor("ctab_out", [CT_OUT, DIM], BF16,
                              kind="ExternalInput")
    usum = nc.dram_tensor("usum", [P, 2], F32, kind="ExternalOutput")
    with tile.TileContext(nc) as tc:
        build_kernel_body(tc, ctxidx.ap(), tnidx.ap(), ctab_in.ap(),
                          ctab_out.ap(), usum.ap(), R=R)
    nc.compile()
    return nc


def _wrap16(arr):
    """flat index list -> [128, n/16] int16 dma_gather layout."""
    w = np.asarray(arr).reshape(-1, 16).T
    return np.tile(w, (8, 1)).astype(np.int16)


def make_in_maps(context, target, negatives, in_emb, out_emb):
    context = np.asarray(context).astype(np.int64)
    target = np.asarray(target).astype(np.int64)
    negatives = np.asarray(negatives).astype(np.int64)
    in_emb = np.asarray(in_emb, dtype=np.float32)
    out_emb = np.asarray(out_emb, dtype=np.float32)
    bf16 = mybir.dt.np(BF16)
    tn_full = np.concatenate([target[:, None], negatives], axis=1)  # [B, 11]
    in_maps = []
    for c in range(NCORES):
        sl = slice(c * BPC, (c + 1) * BPC)
        # [P, NTILES, slots] index cubes (partition = batch row % 128)
        ctx_t = (
            context[sl].reshape(NTILES, P, CWIN).transpose(1, 0, 2)
        )  # [P, T, 10]
        tn_t = (
            tn_full[sl].reshape(NTILES, P, K + 1).transpose(1, 0, 2)
        )  # [P, T, 11], slot 0 = target

        # per-(core, table) compaction
        uin, cin = np.unique(ctx_t, return_inverse=True)
        cin = cin.reshape(ctx_t.shape).astype(np.int32)
        uout, ctn = np.unique(tn_t, return_inverse=True)
        ctn = ctn.reshape(tn_t.shape).astype(np.int32)
        assert len(uin) <= CT_IN and len(uout) <= CT_OUT
        ctab_in = np.zeros((CT_IN, DIM), dtype=bf16)
        ctab_in[: len(uin)] = in_emb[uin].astype(bf16)
        ctab_out = np.zeros((CT_OUT, DIM), dtype=bf16)
        ctab_out[: len(uout)] = out_emb[uout].astype(bf16)

        # slot-major gather lists: position j*2048 + t*128 + p
        ctx_list = cin.transpose(2, 1, 0).reshape(-1)   # [20480]
        tn_list = ctn.transpose(2, 1, 0).reshape(-1)    # [22528]
        ctxidx = np.concatenate(
            [_wrap16(ctx_list[c2 * CHUNK:(c2 + 1) * CHUNK])
             for c2 in range(NCH_CTX)], axis=1)
        tn_parts = []
        for c2 in range(NCH_TN):
            n = min(CHUNK, TN_N - c2 * CHUNK)
            tn_parts.append(_wrap16(tn_list[c2 * CHUNK:c2 * CHUNK + n]))
        tnidx = np.concatenate(tn_parts, axis=1)

        in_maps.append(
            {
                "ctxidx": np.ascontiguousarray(ctxidx),
                "tnidx": np.ascontiguousarray(tnidx),
                "ctab_in": ctab_in,
                "ctab_out": ctab_out,
            }
        )
    return in_maps


_NC_CACHE = []
LAST_RESULT = None  # BassKernelResults of the most recent run (for profiling)


def kernel(**inputs) -> np.ndarray:
    global LAST_RESULT
    in_maps = make_in_maps(
        inputs["context"],
        inputs["target"],
        inputs["negatives"],
        inputs["in_emb"],
        inputs["out_emb"],
    )
    if not _NC_CACHE:
        _NC_CACHE.append(build_nc())
    nc = _NC_CACHE[0]
    res = run_bass_kernel_spmd(nc, in_maps, core_ids=list(range(NCORES)))
    LAST_RESULT = res
    total = 0.0
    for r in res.results:
        u = r["usum"].astype(np.float64)
        total += u[:, 0].sum() + 0.1 * u[:, 1].sum()
    return np.array(-total / B, dtype=np.float32)


# revision 14
# speedup vs baseline: 24.4207x; 13.9599x over previous
"""CBOW negative-sampling loss kernel for 8 TRN2 NeuronCores.

Strategy (data-parallel, per sharding hint):
  - Shard the batch (B=16384) across 8 cores -> 2048 rows/core.
  - Per core the embedding tables are COMPACTED on host: only the
    distinct rows this core's lookups touch (<=20480 for in_emb,
    <=22528 for out_emb) are uploaded, relabelled 0..n-1, bf16.
    Indices fit int16.
  - ALL 43008 row-gathers per core go through batched dma_gather
    ucode in 11 large chunks (4096/2048 indices each) spread
    round-robin over 4 SWDGE queues.  Measured on HW: 4-queue
    dma_gather sustains the full 43008-row / 11 MB gather in ~30 us
    (HBM roofline), vs ~285 us single-queue (descriptor-gen bound)
    and ~440 us for the per-128-row indirect_dma_start path the
    previous version used.
  - Gather lists are SLOT-MAJOR: ctx chunk c carries context slots
    {2c, 2c+1} for all 16 tiles; tn chunk c carries candidates
    {2c, 2c+1} (candidate 0 = target).  Block layout per chunk is
    [local_slot(2) x tile(16) x dim(128)], so the context-sum tree
    and the per-candidate score reductions each run as a handful of
    2048..4096-column DVE instructions (no per-tile small ops).
  - DVE computes the 10-way context sum with a 9-add binary tree in
    bf16, then per candidate k one 2048-col multiply + one segmented
    f32 reduce -> s_all[P, (k,t)].
  - ACT applies sigmoid to ALL scores with scale -0.1 (folds the
    1/10 context-mean normalisation and the negation) and a single
    ln(+eps) with free-dim accumulation.  The target's positive term
    is recovered on host via log sig(x) - log sig(-x) = x, i.e.
    loss row-sum = sum_c log(sig(-s_c/10)+eps) + s_pos/10.
"""

import numpy as np

import concourse.bacc as bacc
import concourse.bass as bass
import concourse.mybir as mybir
import concourse.tile as tile
from concourse.bass_utils import run_bass_kernel_spmd

VOCAB = 100000
DIM = 128
B = 16384
CWIN = 10
K = 10
EPS = 1e-9
NCORES = 8
P = 128
BPC = B // NCORES            # 2048 batch rows per core
NTILES = BPC // P            # 16 tiles of 128 rows
CT_IN = 20480                # compacted in_emb rows (= 2048*10 worst case)
CT_OUT = 22528               # compacted out_emb rows (= 2048*11 worst case)
NQUEUES = 4

CTX_N = BPC * CWIN           # 20480 ctx lookups per core
TN_N = BPC * (K + 1)         # 22528 target+negative lookups per core
CHUNK = 4096                 # gather-list chunk (= 2 slots x 2048 rows)
NCH_CTX = CTX_N // CHUNK     # 5
NCH_TN = (TN_N + CHUNK - 1) // CHUNK   # 6 (last chunk is 2048)

F32 = mybir.dt.float32
BF16 = mybir.dt.bfloat16
I16 = mybir.dt.int16
MULT = mybir.AluOpType.mult
ADD = mybir.AluOpType.add
AX_X = mybir.AxisListType.X
SIGMOID = mybir.ActivationFunctionType.Sigmoid
LN = mybir.ActivationFunctionType.Ln


def build_kernel_body(tc, ctxidx, tnidx, ctab_in, ctab_out, usum, R=1):
    """Emit the per-core program.

    ctxidx: [P, NCH_CTX*256] int16 wrapped dma_gather lists; chunk c
            (columns 256c..256c+255) gathers context slots {2c, 2c+1}
            of all 16 tiles from ctab_in: list position
            j'*2048 + t*128 + p  ->  cin[p, t, 2c+j'].
    tnidx:  [P, 1408] int16; chunk c gathers candidates {2c, 2c+1}
            (c=5: candidate 10 only) from ctab_out; candidate 0 is
            the target.
    usum:   [P, 2] f32; col 0 = sum over candidates/tiles of
            log(sig(-s/10)+eps); col 1 = sum over tiles of raw
            target score (context-SUM dot target, no 1/10).
    """
    nc = tc.nc
    with (
        tc.tile_pool(name="io", bufs=1) as io_pool,
        tc.tile_pool(name="g", bufs=1) as gpool,
        tc.tile_pool(name="w", bufs=1) as wpool,
    ):
        ctxidx_t = io_pool.tile([P, NCH_CTX * (CHUNK // 16)], I16)
        nc.sync.dma_start(out=ctxidx_t[:], in_=ctxidx[:, :])
        tnidx_t = io_pool.tile([P, (TN_N // 16)], I16)
        nc.sync.dma_start(out=tnidx_t[:], in_=tnidx[:, :])

        eps_t = io_pool.tile([P, 1], F32)
        nc.vector.memset(eps_t[:], EPS)

        s_all = io_pool.tile([P, NTILES * (K + 1)], F32)
        us = io_pool.tile([P, 2], F32)

        for r in range(R):
            # ---- gathers: 11 chunks, round-robin over 4 queues ----
            q = 0
            ctx_g = []
            for c in range(NCH_CTX):
                t = gpool.tile([P, (CHUNK // P) * DIM], BF16, tag=f"gc{c}")
                nc.gpsimd.dma_gather(
                    out_ap=t[:].rearrange("p (q d) -> p q d", d=DIM),
                    in_ap=ctab_in[:, :],
                    idxs_ap=ctxidx_t[
                        :, c * (CHUNK // 16):(c + 1) * (CHUNK // 16)
                    ],
                    num_idxs=CHUNK,
                    num_idxs_reg=CHUNK,
                    elem_size=DIM,
                    single_packet=False,
                    queue_num=q % NQUEUES,
                )
                q += 1
                ctx_g.append(t)
            tn_g = []
            for c in range(NCH_TN):
                n = min(CHUNK, TN_N - c * CHUNK)
                t = gpool.tile([P, (n // P) * DIM], BF16, tag=f"gt{c}")
                nc.gpsimd.dma_gather(
                    out_ap=t[:].rearrange("p (q d) -> p q d", d=DIM),
                    in_ap=ctab_out[:, :],
                    idxs_ap=tnidx_t[:, c * (CHUNK // 16):
                                    c * (CHUNK // 16) + n // 16],
                    num_idxs=n,
                    num_idxs_reg=n,
                    elem_size=DIM,
                    single_packet=False,
                    queue_num=q % NQUEUES,
                )
                q += 1
                tn_g.append(t)

            # ---- context-sum tree (9 adds, bf16, big spans) ----
            W = NTILES * DIM  # 2048 cols per slot
            t1 = []
            for c in range(NCH_CTX):
                t = wpool.tile([P, W], BF16, tag=f"t1{c}")
                nc.vector.tensor_add(t[:], ctx_g[c][:, 0:W], ctx_g[c][:, W:2 * W])
                t1.append(t)
            t2a = wpool.tile([P, W], BF16, tag="t2a")
            nc.vector.tensor_add(t2a[:], t1[0][:], t1[1][:])
            t2b = wpool.tile([P, W], BF16, tag="t2b")
            nc.vector.tensor_add(t2b[:], t1[2][:], t1[3][:])
            cs = wpool.tile([P, W], BF16, tag="cs")
            nc.vector.tensor_add(cs[:], t2a[:], t2b[:])
            nc.vector.tensor_add(cs[:], cs[:], t1[4][:])
            cs3 = cs[:].rearrange("p (t d) -> p t d", d=DIM)

            # ---- scores: two k-batches; per batch: per-k 2048-col mult
            # into a shared product buffer, then 3 levels of 2x-eligible
            # interleaved fold-adds halving the per-(k,t) dim width
            # 128 -> 16, then one short 1x segmented reduce.
            KB = [list(range(0, 6)), list(range(6, K + 1))]
            prod = wpool.tile([P, 6 * W], BF16, tag="prod")
            f1 = wpool.tile([P, 6 * W // 2], BF16, tag="f1")
            f2 = wpool.tile([P, 6 * W // 4], BF16, tag="f2")
            f3 = wpool.tile([P, 6 * W // 8], BF16, tag="f3")
            for kb in KB:
                nk = len(kb)
                for i, k in enumerate(kb):
                    src = tn_g[k // 2][:, (k % 2) * W:(k % 2) * W + W]
                    nc.vector.tensor_tensor(
                        prod[:, i * W:(i + 1) * W].rearrange(
                            "p (t d) -> p t d", d=DIM),
                        src.rearrange("p (t d) -> p t d", d=DIM),
                        cs3, MULT,
                    )
                # fold 128 -> 64 -> 32 -> 16 per (k, t) block
                for (src_t, dst_t, w) in (
                    (prod, f1, DIM), (f1, f2, DIM // 2), (f2, f3, DIM // 4),
                ):
                    s4 = src_t[:, 0:nk * NTILES * w].rearrange(
                        "p (b two d) -> p b two d", two=2, d=w // 2)
                    d4 = dst_t[:, 0:nk * NTILES * (w // 2)].rearrange(
                        "p (b one d) -> p b one d", one=1, d=w // 2)
                    nc.vector.tensor_add(d4, s4[:, :, 0:1], s4[:, :, 1:2])
                nc.vector.tensor_reduce(
                    out=s_all[:, kb[0] * NTILES:(kb[-1] + 1) * NTILES],
                    in_=f3[:, 0:nk * NTILES * (DIM // 8)].rearrange(
                        "p (s d) -> p s d", d=DIM // 8),
                    axis=AX_X, op=ADD,
                )

            # ---- target raw-score row sum ------------------------
            nc.vector.tensor_reduce(
                out=us[:, 1:2], in_=s_all[:, 0:NTILES], axis=AX_X, op=ADD,
            )
            # ---- batched activation (one table load each) --------
            sig = io_pool.tile([P, NTILES * (K + 1)], F32)
            nc.scalar.activation(sig[:], s_all[:], SIGMOID, scale=-0.1)
            lnv = io_pool.tile([P, NTILES * (K + 1)], F32)
            nc.scalar.activation(
                lnv[:], sig[:], LN, bias=eps_t[:], accum_out=us[:, 0:1]
            )

        nc.sync.dma_start(out=usum[:, :], in_=us[:])


def build_nc(R=1):
    nc = bacc.Bacc(
        "TRN2",
        target_bir_lowering=False,
        debug=False,
        enable_asserts=False,
        num_devices=NCORES,
        num_swdge_queues=NQUEUES,
    )
    ctxidx = nc.dram_tensor(
        "ctxidx", [P, CTX_N // 16], I16, kind="ExternalInput"
    )
    tnidx = nc.dram_tensor(
        "tnidx", [P, TN_N // 16], I16, kind="ExternalInput"
    )
    ctab_in = nc.dram_tensor("ctab_in", [CT_IN, DIM], BF16,
                             kind="ExternalInput")
    ctab_out = nc.dram_tensor("ctab_out", [CT_OUT, DIM], BF16,
                              kind="ExternalInput")
    usum = nc.dram_tensor("usum", [P, 2], F32, kind="ExternalOutput")
    with tile.TileContext(nc) as tc:
        build_kernel_body(tc, ctxidx.ap(), tnidx.ap(), ctab_in.ap(),
                          ctab_out.ap(), usum.ap(), R=R)
    nc.compile()
    return nc


def _wrap16(arr):
    """flat index list -> [128, n/16] int16 dma_gather layout."""
    w = np.asarray(arr).reshape(-1, 16).T
    return np.tile(w, (8, 1)).astype(np.int16)


def make_in_maps(context, target, negatives, in_emb, out_emb):
    context = np.asarray(context).astype(np.int64)
    target = np.asarray(target).astype(np.int64)
    negatives = np.asarray(negatives).astype(np.int64)
    in_emb = np.asarray(in_emb, dtype=np.float32)
    out_emb = np.asarray(out_emb, dtype=np.float32)
    bf16 = mybir.dt.np(BF16)
    tn_full = np.concatenate([target[:, None], negatives], axis=1)  # [B, 11]
    in_maps = []
    for c in range(NCORES):
        sl = slice(c * BPC, (c + 1) * BPC)
        # [P, NTILES, slots] index cubes (partition = batch row % 128)
        ctx_t = (
            context[sl].reshape(NTILES, P, CWIN).transpose(1, 0, 2)
        )  # [P, T, 10]
        tn_t = (
            tn_full[sl].reshape(NTILES, P, K + 1).transpose(1, 0, 2)
        )  # [P, T, 11], slot 0 = target

        # per-(core, table) compaction
        uin, cin = np.unique(ctx_t, return_inverse=True)
        cin = cin.reshape(ctx_t.shape).astype(np.int32)
        uout, ctn = np.unique(tn_t, return_inverse=True)
        ctn = ctn.reshape(tn_t.shape).astype(np.int32)
        assert len(uin) <= CT_IN and len(uout) <= CT_OUT
        ctab_in = np.zeros((CT_IN, DIM), dtype=bf16)
        ctab_in[: len(uin)] = in_emb[uin].astype(bf16)
        ctab_out = np.zeros((CT_OUT, DIM), dtype=bf16)
        ctab_out[: len(uout)] = out_emb[uout].astype(bf16)

        # slot-major gather lists: position j*2048 + t*128 + p
        ctx_list = cin.transpose(2, 1, 0).reshape(-1)   # [20480]
        tn_list = ctn.transpose(2, 1, 0).reshape(-1)    # [22528]
        ctxidx = np.concatenate(
            [_wrap16(ctx_list[c2 * CHUNK:(c2 + 1) * CHUNK])
             for c2 in range(NCH_CTX)], axis=1)
        tn_parts = []
        for c2 in range(NCH_TN):
            n = min(CHUNK, TN_N - c2 * CHUNK)
            tn_parts.append(_wrap16(tn_list[c2 * CHUNK:c2 * CHUNK + n]))
        tnidx = np.concatenate(tn_parts, axis=1)

        in_maps.append(
            {
                "ctxidx": np.ascontiguousarray(ctxidx),
                "tnidx": np.ascontiguousarray(tnidx),
                "ctab_in": ctab_in,
                "ctab_out": ctab_out,
            }
        )
    return in_maps


_NC_CACHE = []
LAST_RESULT = None  # BassKernelResults of the most recent run (for profiling)


def kernel(**inputs) -> np.ndarray:
    global LAST_RESULT
    in_maps = make_in_maps(
        inputs["context"],
        inputs["target"],
        inputs["negatives"],
        inputs["in_emb"],
        inputs["out_emb"],
    )
    if not _NC_CACHE:
        _NC_CACHE.append(build_nc())
    nc = _NC_CACHE[0]
    res = run_bass_kernel_spmd(nc, in_maps, core_ids=list(range(NCORES)))
    LAST_RESULT = res
    total = 0.0
    for r in res.results:
        u = r["usum"].astype(np.float64)
        total += u[:, 0].sum() + 0.1 * u[:, 1].sum()
    return np.array(-total / B, dtype=np.float32)
